# revision 16
# baseline (speedup 1.0000x reference)
"""Mistral-style MHA prefill kernel for Trainium2, 8-way tensor-parallel over heads.

Problem (hardcoded): B=1, S=2048, DIM=4096, 32 q-heads / 8 kv-heads, head_dim=128,
sliding window 2048 (== S, so the mask is exactly causal), rope theta 1e4.

Sharding: core c owns q-heads [4c, 4c+4) and kv-head c. wq/wk/wv are sharded on the
head axis, wo on its input (head) axis; each core computes a full-shape partial
output and the host sums the 8 partials (row-parallel linear + host all-reduce).

Design (~400 us vs the fp32r baseline's ~520 us; measured rel err 4.4e-3):
  - All matmul operands are bf16 (PSUM accumulation stays fp32). Same PE rate
    as float32r (1 cyc/row) but halves DMA traffic and unlocks the 2x/4x DVE
    perf modes for 2-byte dtypes.
  - Host pre-shuffles every input into its exact SBUF layout ([128, free]) so
    loads are few, huge, contiguous-row DMAs: per-transfer fixed cost (~2.8us)
    dominates small transfers. The DMA bus fair-shares among outstanding
    transfers, so arrival ORDER is controlled by limiting what is in flight
    (x-pool demand pacing; w stream paced by 1-elem copies of arrived x).
  - The 160 softmax-denominator matmuls are off the PE: e-tiles accumulate
    elementwise into esum on the DVE (bf16 2x), one ones-matmul per
    (head, block) reduces esum across partitions (broadcast for free), and
    reciprocal_approx_fast (~0.7us vs reciprocal's 3.4us) inverts it. The
    whole tail is deferred into the next block's score stream.
  - Diagonal k-tiles run scores/exp/mask/esum/PV on their valid suffix
    [t*128:] only (saves ~15us ACT + ~10us PE; the dead prefix is never
    read).
  - Attention alone is ACT-bound (exp 607 ns/tile vs 426 ns PE), so the
    output projection of q-block b-1 is software-pipelined INTO the attention
    stream of q-block b one matmul at a time; out-proj of block 3 forms a
    PE-dense tail. j-major unit order + 2/1-buffered pw PSUM tags keep the
    single PSUM-bank budget (sp3 + ot2 + pw3 = 8) stall-free; the deferred
    dn matmul borrows the pw0 rotation so all 3 sp slots stay with the
    score pipeline.
  - The PE ramps 0.65->1.2->2.4 GHz with ~3us of continuous busy (p-state):
    throwaway zero matmuls warm it up during the initial DMA wait, and every
    removed bubble also speeds up the matmuls after it.
  - V transposes are emitted one s-block late so they never block the
    in-order Tensor queue on the ident constant / vt eviction.
  - Block b=0's scores/exp/mask/esum are hoisted INTO s-block 3's QKV
    stream (they only need s-block 0's qt/kt, and one spare PSUM bank
    exists there): phase 2 opens with a PV-only b=0, so the attention
    phase never starts ACT-bound.
  - Layouts as baseline: x pre-transposed, per-quadrant rope permutation with
    stream_shuffle +-16, sqrt(scale) folded into the rope tables, transposed
    scores S_T[k, q], causality at (k-tile, 512-q-block) granularity, diagonal
    masked with a 128-wide triangle tile.
"""

import numpy as np

B = 1
S = 2048
DIM = 4096
N_HEADS = 32
N_KV = 8
DH = 128
NCORES = 8
HPC = N_HEADS // NCORES  # q heads per core
FQKV = HPC * DH + 2 * DH  # 768 projection rows per core
NKT = S // DH  # 16 k tiles
NQB = S // 512  # 4 q blocks
NDCH = DIM // DH  # 32 contraction chunks

_PROGRAM = None

# stream_shuffle mask: swap 16-partition halves within each 32-partition quadrant
_SWAP16 = [(i + 16) % 32 for i in range(32)]


def _head_perm():
    """Permutation of head_dim rows: quadrant q holds [re_16q..re_16q+15,
    im_16q..im_16q+15], so RoPE pairs are +-16 apart within a quadrant."""
    p = np.empty(DH, dtype=np.int64)
    for row in range(DH):
        q, j = divmod(row, 32)
        i = 16 * q + (j % 16)  # rope pair index
        p[row] = 2 * i + (0 if j < 16 else 1)
    return p


def _build_program():
    import concourse.bacc as bacc
    import concourse.mybir as mybir
    import concourse.tile as tile

    F32 = mybir.dt.float32
    BF16 = mybir.dt.bfloat16
    EXP = mybir.ActivationFunctionType.Exp

    nc = bacc.Bacc("TRN2", target_bir_lowering=False, debug=False,
                   enable_asserts=False)

    # All inputs are pre-shuffled on the host into the EXACT SBUF layout
    # ([128 partitions, free]) so every load is one huge contiguous-row DMA:
    # per-transfer fixed overhead (~2.8 us) and per-queue issue pace
    # (~1.4 us/transfer) dominate small transfers, so few+big wins.
    x2_d = nc.dram_tensor("x2", [DH, NQB * NDCH * 512], BF16,
                          kind="ExternalInput")
    wq2_d = nc.dram_tensor("wq2", [DH, NDCH * FQKV], BF16,
                           kind="ExternalInput")
    wo2_d = nc.dram_tensor("wo2", [DH, HPC * DIM], BF16, kind="ExternalInput")
    # consts pack: csA | csB | tri512 | ident | ones128
    CPK = 2 * S + 512 + 2 * DH
    cpk_d = nc.dram_tensor("cpk", [DH, CPK], BF16, kind="ExternalInput")
    sign_d = nc.dram_tensor("sign", [DH, 1], F32, kind="ExternalInput")
    # out stored as bf16: halves store wire time (33.5MB -> 16.8MB per core,
    # the tail's last-block stores were draining ~10us past the final matmul).
    # Host sums the 8 partials in f32; each partial is ~1/sqrt(8) of the total
    # so the bf16 rounding adds only ~0.2% rms.
    out_d = nc.dram_tensor("out", [S, DIM], BF16, kind="ExternalOutput")

    with tile.TileContext(nc) as tc:
        with (
            tc.tile_pool(name="consts", bufs=1) as cpool,
            tc.tile_pool(name="persist", bufs=1) as ppool,
            tc.tile_pool(name="xin", bufs=3) as xpool,
            tc.tile_pool(name="ropet", bufs=2) as rtp,
            tc.tile_pool(name="rawsb", bufs=5) as rawpool,
            tc.tile_pool(name="vtt", bufs=1) as vtp,
            tc.tile_pool(name="warm", bufs=1) as wzpool,
        ):
            cpk_sb = cpool.tile([DH, CPK], BF16)
            csA_sb = cpk_sb[:, 0:S]
            csB_sb = cpk_sb[:, S:2 * S]
            tri512_sb = cpk_sb[:, 2 * S:2 * S + 512]
            ident_sb = cpk_sb[:, 2 * S + 512:2 * S + 512 + DH]
            ones128_sb = cpk_sb[:, 2 * S + 512 + DH:2 * S + 512 + 2 * DH]
            sign_sb = cpool.tile([DH, 1], F32)

            qt = [ppool.tile([DH, S], BF16, name=f"qt{h}") for h in range(HPC)]
            kt = ppool.tile([DH, S], BF16)
            vn = ppool.tile([DH, S], BF16)  # V normal layout, 16 [128,128] chunks
            wo_sb = ppool.tile([DH, HPC * DIM], BF16)
            # b=0 attention scores/exp are hoisted into s-block 3's QKV
            # stream (they only need s-block 0's qt/kt): e-tiles and esums
            # persist here until phase 2 runs the PV-only b=0 blocks
            e0_sb = ppool.tile([DH, HPC * NQB * 512], BF16)
            es0_sb = ppool.tile([DH, HPC * 512], BF16)
            # otn aliases qt: attention block b is the last reader of
            # qt[h][:, b*512:(b+1)*512], so the normalized out^T overwrites it.
            otn = qt

            def emit_rope(f, sb_i, raw):
                # head_dim permuted so pairs sit +-16 apart within each
                # 32-partition quadrant: dest = p1 + sign*p3 where
                # p1 = q*cos, p3 = halfswap(q)*sin. All bf16 SBUF operands so
                # the muls run 2x and the stt 4x on the DVE.
                col = slice(sb_i * 512, (sb_i + 1) * 512)
                dest = qt[f] if f < HPC else kt
                qs_t = rtp.tile([DH, 512], BF16, name="qs_t", tag="qs")
                p1 = rtp.tile([DH, 512], BF16, name="p1", tag="p1")
                nc.vector.stream_shuffle(qs_t[:], raw[:], _SWAP16)
                nc.vector.tensor_mul(p1[:], raw[:], csA_sb[:, col])
                nc.vector.tensor_mul(qs_t[:], qs_t[:], csB_sb[:, col])
                nc.vector.scalar_tensor_tensor(
                    dest[:, col], qs_t[:], sign_sb[:], p1[:],
                    mybir.AluOpType.mult, mybir.AluOpType.add)

            # ---------------- Phase 1: QKV projections --------------------
            with (
                tc.tile_pool(name="mps", bufs=6, space="PSUM") as mps,
                tc.tile_pool(name="trps", bufs=1, space="PSUM") as trps,
                tc.tile_pool(name="sp0ps", bufs=1, space="PSUM") as sp0ps,
                tc.tile_pool(name="wsb", bufs=1) as wpool,
            ):
                w_sb = wpool.tile([DH, NDCH * FQKV], BF16)

                # PE p-state warmup: the Tensor engine ramps 0.65 -> 1.2 ->
                # 2.4 GHz over ~3us of continuous execution. Run throwaway
                # zero matmuls while the first w/x DMAs are in flight so the
                # real matmuls start at full clock.
                wz = wzpool.tile([DH, 512], BF16)
                nc.vector.memset(wz[:], 0)
                wps = mps.tile([DH, 512], F32, name="wps", tag="ps")
                for i in range(10):
                    nc.tensor.matmul(wps[:], wz[:, 0:DH], wz[:],
                                     start=(i == 0), stop=(i == 9))

                def emit_score0(h, k):
                    # hoisted scores/exp/mask/esum for block (h, b=0); all 4
                    # of its k-tiles are diagonal (t = k), suffix [k*128:]
                    lo = k * DH
                    o = (h * NQB + k) * 512
                    sp = sp0ps.tile([DH, 512], F32, name="sp0", tag="sp0")
                    nc.tensor.matmul(
                        sp[:, lo:], kt[:, k * DH:(k + 1) * DH],
                        qt[h][:, lo:512], start=True, stop=True)
                    nc.scalar.activation(e0_sb[:, o + lo:o + 512],
                                         sp[:, lo:], EXP)
                    nc.vector.tensor_mul(
                        e0_sb[:, o + lo:o + lo + DH],
                        e0_sb[:, o + lo:o + lo + DH], tri512_sb[:, 512 - DH:])
                    es0 = slice(h * 512, (h + 1) * 512)
                    if k == 0:
                        nc.vector.tensor_copy(es0_sb[:, es0],
                                              e0_sb[:, o:o + 512])
                    else:
                        nc.vector.tensor_add(
                            es0_sb[:, h * 512 + lo:(h + 1) * 512],
                            es0_sb[:, h * 512 + lo:(h + 1) * 512],
                            e0_sb[:, o + lo:o + 512])

                def emit_sblock(sb_i):
                    # x consumed in multi-chunk groups: one DMA per group
                    # (contiguous in the host-shuffled x2 layout), alternating
                    # the SP-HWDGE and SWDGE rings. The DMA bus fair-shares
                    # among outstanding transfers, so arrival order is set by
                    # limiting what's in flight: the x pool (bufs=2) demand-
                    # paces the x groups, and s-block 0's w groups ride the
                    # SAME queues right behind their x group, inheriting that
                    # pacing. Only the first two small w groups go eagerly on
                    # the otherwise-idle ACT ring.
                    groups = ([(0, 2), (2, 4), (4, 8), (8, 12), (12, 16),
                               (16, 20), (20, 24), (24, 28), (28, 32)]
                              if sb_i == 0 else
                              [(0, 8), (8, 16), (16, 24), (24, 32)])
                    # w stream schedule for s-block 0, paced by 1-elem copies
                    # of arrived x groups. The DMA bus fair-shares among ALL
                    # in-flight transfers (~213GB/s effective), so a batch of
                    # late transfers finishes together and LATE; the late
                    # window (gi>=6) therefore rides one queue in strict need
                    # order (x then w per 4-chunk group) with ~2 batches in
                    # flight.
                    wsched = {0: [(0, 2)], 1: [(2, 4)], 2: [(4, 8)],
                              3: [(8, 12)], 4: [(12, 16)], 5: [(16, 20)],
                              6: [(20, 24)], 7: [(24, 28)], 8: [(28, 32)]}
                    ps = [mps.tile([DH, 512], F32, name=f"ps{f}", tag="ps")
                          for f in range(6)]
                    xgs = []
                    for gi, (a, b) in enumerate(groups):
                        xg = xpool.tile([DH, (b - a) * 512], BF16, name="xg",
                                        tag="xg")
                        late = sb_i == 0 and gi >= 6
                        xeng = (nc.scalar if late else
                                nc.sync if (sb_i + gi) % 2 == 0 else
                                nc.gpsimd)
                        xo = sb_i * NDCH * 512
                        xgs.append(xg)
                        if sb_i == 0 and gi >= 2:
                            # demand-pace: a 1-elem copy of an arrived x
                            # group gates this batch, so the fair-shared
                            # bus serves batches in need order instead of
                            # all at once
                            back = 3 if gi >= 5 else 2
                            nc.scalar.copy(wz[0:1, 0:1],
                                           xgs[-back][0:1, 0:1])
                        xeng.dma_start(xg[:],
                                       x2_d[:, xo + a * 512:xo + b * 512])
                        if sb_i == 0:
                            for wa, wb in wsched.get(gi, []):
                                nc.scalar.dma_start(
                                    w_sb[:, wa * FQKV:wb * FQKV],
                                    wq2_d[:, wa * FQKV:wb * FQKV])
                            if gi == 2:
                                # small consts (tri/ident/ones, 0.2MB) early:
                                # first read is s-block 0's V transposes
                                nc.scalar.dma_start(cpk_sb[:, 2 * S:],
                                                    cpk_d[:, 2 * S:])
                                nc.scalar.dma_start(sign_sb[:], sign_d[:])
                            if gi == 8:
                                # rope tables (1MB) last in need order; first
                                # read is rope at the end of s-block 0 (~57us)
                                nc.scalar.dma_start(cpk_sb[:, 0:2 * S],
                                                    cpk_d[:, 0:2 * S])
                        if sb_i == 2 and gi < 2:
                            # wo (4MB bf16) in two halves on the ACT ring;
                            # deferred to s-block 2 so it never steals DMA
                            # bus from the s-block 0/1 x and w prefetch
                            half = HPC * DIM // 2
                            nc.scalar.dma_start(
                                wo_sb[:, gi * half:(gi + 1) * half],
                                wo2_d[:, gi * half:(gi + 1) * half])
                        for d in range(a, b):
                            xcol = slice((d - a) * 512, (d - a + 1) * 512)
                            for f in range(6):
                                nc.tensor.matmul(
                                    ps[f][:],
                                    w_sb[:, d * FQKV + f * DH:
                                         d * FQKV + (f + 1) * DH],
                                    xg[:, xcol], start=(d == 0),
                                    stop=(d == NDCH - 1))
                            if sb_i == NQB - 1 and d % 2 == 1:
                                # one hoisted b=0 score step per 2 chunks:
                                # the QKV matmuls are the PE filler for the
                                # exp latency, and the exps land on s-block
                                # 3's idle ACT, so phase 2's b=0 becomes
                                # PV-only (no ACT dependency at all)
                                emit_score0(*divmod(d // 2, NQB))
                        if gi == 0 and prev_vt is not None:
                            # previous s-block's V transposes, emitted mid-
                            # stream: if they led the next s-block they would
                            # block the in-order Tensor queue on the ident
                            # constant / vt eviction
                            emit_vtr(sb_i - 1, prev_vt)
                    # Fast raw PSUM->SBUF evictions (alternating ACT/DVE) free
                    # the accumulator banks quickly; RoPE runs later from SBUF.
                    vt_t = vtp.tile([DH, 512], BF16, name="vt_t", tag="vt")
                    nc.scalar.copy(vt_t[:], ps[5][:])
                    raws = {}
                    for i, f in enumerate([0, 4, 2, 1, 3]):
                        raw = rawpool.tile([DH, 512], BF16, name="raw", tag="raw")
                        raws[f] = raw
                        if i % 2 == 1:
                            nc.scalar.copy(raw[:], ps[f][:])
                        else:
                            nc.vector.tensor_copy(raw[:], ps[f][:])
                    return raws, vt_t

                def emit_vtr(sb_i, vt_t):
                    for t in range(4):
                        tp = trps.tile([DH, DH], BF16, name="tp", tag="tp")
                        nc.tensor.transpose(tp[:], vt_t[:, t * DH:(t + 1) * DH],
                                            ident_sb[:])
                        j = sb_i * 4 + t
                        nc.vector.tensor_copy(vn[:, j * DH:(j + 1) * DH], tp[:])

                prev_vt = None
                for sb_i in range(NQB):
                    raws, prev_vt = emit_sblock(sb_i)
                    if sb_i < NQB - 1:
                        for f in [0, 4, 1, 2, 3]:
                            emit_rope(f, sb_i, raws[f])
                # boundary warmup: the phase handoff stalls the PE ~2.5us
                # waiting for s-block 3's accumulator evictions to free a
                # phase-2 PSUM bank — which also resets the p-state ramp.
                # Zero matmuls into the now-idle sp0 bank have no deps, so
                # they fill the window and keep the clock at 2.4 GHz.
                wps2 = sp0ps.tile([DH, 512], F32, name="wps2", tag="sp0")
                for i in range(14):
                    nc.tensor.matmul(wps2[:], wz[:, 0:DH], wz[:],
                                     start=(i == 0), stop=(i == 13))

            # ---------------- Phase 2: attention + out-proj pipelined ------
            with (
                # otps declared first: its banks then map to the earliest-
                # evicted s-block 3 accumulators, so phase 2's opening PV
                # (which only needs an ot bank) starts sooner after the
                # phase boundary
                tc.tile_pool(name="otps", bufs=2, space="PSUM") as otps,
                tc.tile_pool(name="pwps", bufs=1, space="PSUM") as pwps,
                tc.tile_pool(name="spps", bufs=3, space="PSUM") as spps,
                tc.tile_pool(name="esb", bufs=4) as epool,
                tc.tile_pool(name="essb", bufs=2) as espool,
                tc.tile_pool(name="bcsb", bufs=2) as bcpool,
                tc.tile_pool(name="evsb", bufs=6) as evpool,
            ):
                class OpjEmitter:
                    """Output projection for s-tiles of q-block bprev, emitted
                    one matmul per step() so the attention emitter can pace it.
                    Unit = (st, dh_i, jj): 8 matmuls (4 heads x 2 adjacent
                    512-wide dout cols, stationary otn[h] shared), 2 PSUM
                    evictions (ACT/DVE), one [128,1024] store. Plain state
                    machine (not a generator): tile-pool allocs from a
                    suspended generator frame break the pool's scope-matched
                    reuse dependencies."""

                    def __init__(self, bprev, u0, dense=False):
                        self.units = [(st, dh_i, jj)
                                      for st in range(4 * bprev, 4 * bprev + 4)
                                      for dh_i in range(2) for jj in range(2)]
                        self.ui = 0
                        self.mi = 0
                        self.u = u0
                        # dense: the final PE-only tail. ACT was the pacing
                        # engine there (2 evicts + store issues ~ PE's 1.73us
                        # per unit, zero slack -> ~3us of stalls); the DVE and
                        # sync queue are idle, so spread: pw0 evict on DVE,
                        # pw1 on ACT, stores on the sync ring.
                        self.dense = dense
                        self.pw0 = self.pw1 = None

                    def step(self):
                        if self.ui >= len(self.units):
                            return False
                        st, dh_i, jj = self.units[self.ui]
                        scol = slice(st * DH, (st + 1) * DH)
                        base = dh_i * 2048 + jj * 1024
                        if self.mi == 0:
                            self.pw0 = pwps.tile([DH, 512], F32, name="pw0",
                                                 tag="pw0", bufs=2)
                            self.pw1 = pwps.tile([DH, 512], F32, name="pw1",
                                                 tag="pw1", bufs=1)
                        # j-major: pw0's accumulation (4 matmuls) completes
                        # first and evicts while pw1's matmuls run; pw1's
                        # next-unit reuse then trails its eviction by 4
                        # matmuls, so a single pw1 buffer never stalls the PE
                        j2, h2 = divmod(self.mi, HPC)
                        o = h2 * DIM + base + j2 * 512
                        pw = self.pw0 if j2 == 0 else self.pw1
                        nc.tensor.matmul(
                            pw[:], otn[h2][:, scol], wo_sb[:, o:o + 512],
                            start=(h2 == 0), stop=(h2 == HPC - 1))
                        self.mi += 1
                        if self.mi == HPC:
                            self.ev = evpool.tile([DH, 1024], BF16, name="ev",
                                                  tag="ev")
                            if self.dense:
                                nc.vector.tensor_copy(self.ev[:, 0:512],
                                                      self.pw0[:])
                            else:
                                # both evicts on ACT: it has slack in every
                                # block (after the diagonal restriction), and
                                # keeping the DVE clear lets each deferred
                                # dn's reciprocal run promptly so its pw0
                                # slot frees
                                nc.scalar.copy(self.ev[:, 0:512], self.pw0[:])
                        if self.mi == 8:
                            ev = self.ev
                            if not self.dense and \
                                    self.ui >= len(self.units) - 2:
                                # final units: DVE is idle here while ACT
                                # still drains eviction backlog — shortens
                                # the last evict->store chain (kernel tail)
                                nc.vector.tensor_copy(ev[:, 512:1024],
                                                      self.pw1[:])
                            else:
                                nc.scalar.copy(ev[:, 512:1024], self.pw1[:])
                            dst = out_d[scol, base:base + 1024]
                            if self.ui >= len(self.units) - 4:
                                # split the final stores across both HWDGE
                                # rings to shrink the kernel tail
                                nc.sync.dma_start(
                                    out_d[scol, base:base + 512], ev[:, 0:512])
                                nc.scalar.dma_start(
                                    out_d[scol, base + 512:base + 1024],
                                    ev[:, 512:1024])
                            elif self.dense:
                                nc.sync.dma_start(dst, ev[:])
                            else:
                                # stores alternate the two HWDGE rings; the
                                # SWDGE (gpsimd) ring is too slow for stores
                                (nc.sync if self.u % 2 == 0 else nc.scalar
                                 ).dma_start(dst, ev[:])
                            self.u += 1
                            self.mi = 0
                            self.ui += 1
                        return True

                prev_tail = [None]

                def emit_block(h, b, fill, last=False, pre=None):
                    cb = slice(b * 512, (b + 1) * 512)
                    nk = 4 * b + 4  # k tiles contributing to this q block
                    ot_b = otps.tile([DH, 512], F32, name="ot", tag="ot")
                    if pre is None:
                        esum = espool.tile([DH, 512], BF16, name="esum",
                                           tag="es")
                        e_tiles = [None] * nk
                    else:
                        e_tiles, esum = pre

                    def emit_scores(k):
                        # diagonal k-tile t: only q >= t*128 is valid, so
                        # scores/exp/mask/esum/PV all run on the suffix
                        # [t*128:] alone — the dead prefix [0, t*128) is
                        # never written OR read by anyone
                        t = k - 4 * b if k // 4 == b else None
                        lo = t * DH if t else 0
                        e = epool.tile([DH, 512], BF16, name="E", tag="E")
                        e_tiles[k] = (e, lo)
                        sp = spps.tile([DH, 512], F32, name="sp", tag="sp")
                        nc.tensor.matmul(
                            sp[:, lo:], kt[:, k * DH:(k + 1) * DH],
                            qt[h][:, b * 512 + lo:(b + 1) * 512],
                            start=True, stop=True)
                        nc.scalar.activation(e[:, lo:], sp[:, lo:], EXP)
                        if t is not None:
                            # triangle mask on the boundary 128 columns
                            nc.vector.tensor_mul(
                                e[:, lo:lo + DH], e[:, lo:lo + DH],
                                tri512_sb[:, 512 - DH:])
                        # accumulate the softmax denominator on the DVE
                        if k == 0:
                            nc.vector.tensor_copy(esum[:], e[:])
                        else:
                            nc.vector.tensor_add(esum[:, lo:], esum[:, lo:],
                                                 e[:, lo:])

                    def emit_pv(k):
                        e, lo = e_tiles[k]
                        st_, sp_ = (k == 0), (k == nk - 1)
                        nc.tensor.matmul(ot_b[:, lo:],
                                         vn[:, k * DH:(k + 1) * DH],
                                         e[:, lo:], start=st_, stop=sp_)

                    # 2-deep software pipeline: scores run two steps ahead of
                    # PV so exp/mask latency never stalls the PE; out-proj
                    # matmuls of block b-1 are interleaved to absorb ACT lag.
                    # With pre (hoisted b=0), it's a PV-only chain.
                    if pre is None:
                        emit_scores(0)
                        if nk > 1:
                            emit_scores(1)
                    # the previous block's normalization tail lands here: by
                    # now its exp->esum chain has long drained, so its
                    # ones-matmul never stalls the PE
                    if prev_tail[0] is not None:
                        prev_tail[0]()
                        prev_tail[0] = None
                    fill(1)
                    for k in range(2, nk):
                        if pre is None:
                            emit_scores(k)
                        emit_pv(k - 2)
                        fill(1)
                    if nk > 1:
                        emit_pv(nk - 2)
                    emit_pv(nk - 1)
                    fill(4)

                    def tail():
                        # denominator: one ones-matmul reduces esum across
                        # partitions (every PSUM row = colsum -> broadcast
                        # for free), then fast reciprocal + fused normalize.
                        # dn borrows the pw0 rotation: keeps all 3 sp
                        # slots for the score pipeline, so the next block's
                        # third scores matmul never waits on an exp
                        dn_b = pwps.tile([DH, 512], F32, name="dn", tag="pw0",
                                         bufs=2)
                        nc.tensor.matmul(dn_b[:], ones128_sb[:], esum[:],
                                         start=True, stop=True)
                        bc_sb = bcpool.tile([DH, 512], F32, name="bc_sb",
                                            tag="bcs")
                        nc.vector.reciprocal_approx_fast(out=bc_sb[:],
                                                         in_=dn_b[:])
                        nc.vector.tensor_mul(otn[h][:, cb], ot_b[:], bc_sb[:])

                    if last:
                        # last head of the q-block: the next block's out-proj
                        # needs this otn, so normalize NOW, padded with filler
                        # matmuls to cover the exp->esum chain latency
                        fill(6)
                        tail()
                    else:
                        # otherwise defer the tail into the next block's
                        # score stream, where its deps have long drained
                        prev_tail[0] = tail

                gen = None

                def fill(n):
                    if gen is None:
                        return
                    for _ in range(n):
                        if not gen.step():
                            break

                # last s-block's RoPE: kt and qt[0] first (needed by b<=3 /
                # b=3 of head 0), the rest spread between early blocks
                emit_rope(4, 3, raws[4])
                emit_rope(0, 3, raws[0])
                for b in range(NQB):
                    gen = OpjEmitter(b - 1, 16 * (b - 1)) if b >= 1 else None
                    for h in range(HPC):
                        if b == 1 and h >= 1:
                            # s-block 3's remaining RoPEs, deferred past the
                            # b=0 tails so the DVE isn't congested right when
                            # otn(:,b=0) must normalize for out-proj(0)
                            emit_rope(h, 3, raws[h])
                        if b == 0 and h == 1:
                            # s-block 3's V transposes: b=0 is PV-only so the
                            # sp pool is idle here, and vn[:, 1536:] isn't
                            # read until the b=3 blocks
                            for t in range(4):
                                tp = spps.tile([DH, DH], BF16, name="tp",
                                               tag="sp")
                                nc.tensor.transpose(
                                    tp[:], prev_vt[:, t * DH:(t + 1) * DH],
                                    ident_sb[:])
                                j = (NQB - 1) * 4 + t
                                nc.vector.tensor_copy(
                                    vn[:, j * DH:(j + 1) * DH], tp[:])
                        pre = None
                        if b == 0:
                            pre = ([(e0_sb[:, (h * NQB + k) * 512:
                                           (h * NQB + k + 1) * 512], k * DH)
                                    for k in range(NQB)],
                                   es0_sb[:, h * 512:(h + 1) * 512])
                        emit_block(h, b, fill, last=(h == HPC - 1), pre=pre)
                    fill(1 << 30)  # drain the rest of block b-1's out-proj
                if prev_tail[0] is not None:
                    prev_tail[0]()
                    prev_tail[0] = None
                gen = OpjEmitter(NQB - 1, 16 * (NQB - 1), dense=True)
                fill(1 << 30)  # PE-dense tail

    nc.compile()
    return nc


def get_program():
    global _PROGRAM
    if _PROGRAM is None:
        _PROGRAM = _build_program()
    return _PROGRAM


def make_in_maps(inputs):
    """Host-side sharding / layout prep. Returns one input dict per core."""
    import ml_dtypes
    bf16 = ml_dtypes.bfloat16

    x = np.asarray(inputs["x"], dtype=np.float32)
    wq = np.asarray(inputs["wq"], dtype=np.float32)
    wk = np.asarray(inputs["wk"], dtype=np.float32)
    wv = np.asarray(inputs["wv"], dtype=np.float32)
    wo = np.asarray(inputs["wo"], dtype=np.float32)
    cos = np.asarray(inputs["freqs_cos"], dtype=np.float32)  # (S, 64)
    sin = np.asarray(inputs["freqs_sin"], dtype=np.float32)

    xT = x.reshape(S, DIM).T  # (DIM, S)
    # x2: SBUF-mirror layout [128, (sb, d, col)] so each x group load is one
    # contiguous-row DMA: x2[p, sb*32*512 + d*512 + c] = xT[d*128+p, sb*512+c]
    x2 = np.ascontiguousarray(
        xT.reshape(NDCH, DH, NQB, 512).transpose(1, 2, 0, 3).reshape(
            DH, NQB * NDCH * 512)).astype(bf16)

    perm = _head_perm()
    sq = np.float32(DH ** -0.25)  # sqrt of 1/sqrt(head_dim), folded into Q and K
    rows = np.arange(DH)
    pair_idx = 16 * (rows // 32) + (rows % 32) % 16
    csA = np.ascontiguousarray(cos.T[pair_idx] * sq)   # (128, S)
    csB = np.ascontiguousarray(sin.T[pair_idx] * sq)
    sign = np.where((rows % 32) < 16, -1.0, 1.0).astype(np.float32).reshape(DH, 1)
    tri = np.triu(np.ones((DH, DH), dtype=np.float32))
    tri512 = np.concatenate([np.zeros((DH, 512 - DH), np.float32), tri], axis=1)
    ident = np.eye(DH, dtype=np.float32)
    ones128 = np.ones((DH, DH), dtype=np.float32)
    # consts pack mirrors the cpk_sb slices: csA | csB | tri512 | ident | ones
    cpk = np.concatenate([csA, csB, tri512, ident, ones128],
                         axis=1).astype(bf16)

    wqh = wq.reshape(N_HEADS, DH, DIM)[:, perm, :]
    wkh = wk.reshape(N_KV, DH, DIM)[:, perm, :]
    wvh = wv.reshape(N_KV, DH, DIM)

    in_maps = []
    for c in range(NCORES):
        w_c = np.concatenate(
            [wqh[HPC * c:HPC * (c + 1)].reshape(HPC * DH, DIM),
             wkh[c], wvh[c]], 0)  # (768, DIM)
        wqkvT = w_c.T  # (DIM, 768)
        # wq2: SBUF-mirror [128, (d, f)]: wq2[p, d*768+f] = wqkvT[d*128+p, f]
        wq2 = np.ascontiguousarray(
            wqkvT.reshape(NDCH, DH, FQKV).transpose(1, 0, 2).reshape(
                DH, NDCH * FQKV)).astype(bf16)
        woT = wo[:, HPC * DH * c:HPC * DH * (c + 1)].T  # (512, DIM)
        # wo2: SBUF-mirror [128, (ch, dout)]
        wo2 = np.ascontiguousarray(
            woT.reshape(HPC, DH, DIM).transpose(1, 0, 2).reshape(
                DH, HPC * DIM)).astype(bf16)
        in_maps.append({
            "x2": x2, "wq2": wq2, "wo2": wo2, "cpk": cpk, "sign": sign,
        })
    return in_maps


def _ensure_ntff_hook():
    """The agent image's `antenv` lacks `axon_hooks`; recreate it so
    run_bass_kernel_spmd(trace=True) can capture NTFF profiles."""
    import sys
    try:
        from antenv.axon_hooks import get_axon_ntff_profile_hook  # noqa: F401
        return
    except ImportError:
        pass
    import contextlib
    import ctypes
    import types

    so_path = "/opt/axon/libaxon_pjrt.so"
    hook = None
    try:
        lib = ctypes.CDLL(so_path)
        if hasattr(lib, "axon_start_nrt_profile"):
            lib.axon_start_nrt_profile.argtypes = [
                ctypes.POINTER(ctypes.c_int64), ctypes.c_size_t]
            lib.axon_start_nrt_profile.restype = ctypes.c_int64
            lib.axon_stop_nrt_profile.argtypes = [ctypes.c_char_p]
            lib.axon_stop_nrt_profile.restype = ctypes.c_int64

            @contextlib.contextmanager
            def _hook(output_dir, device_ids):
                import jax
                jax.devices()
                if device_ids:
                    ids = (ctypes.c_int64 * len(device_ids))(*device_ids)
                    rc = lib.axon_start_nrt_profile(ids, len(device_ids))
                else:
                    rc = lib.axon_start_nrt_profile(None, 0)
                if rc != 0:
                    raise RuntimeError(f"axon_start_nrt_profile rc={rc}")
                try:
                    yield
                finally:
                    n = lib.axon_stop_nrt_profile(str(output_dir).encode())
                    print(f"profile: {n} file(s) written to {output_dir}")

            hook = _hook
    except OSError:
        pass

    mod = types.ModuleType("antenv.axon_hooks")
    mod._hook = hook
    mod.get_axon_ntff_profile_hook = lambda: mod._hook
    mod.set_axon_ntff_profile_hook = lambda h: setattr(mod, "_hook", h)
    sys.modules["antenv.axon_hooks"] = mod


def run(inputs, trace=False):
    from concourse.bass_utils import run_bass_kernel_spmd
    if trace:
        _ensure_ntff_hook()
    nc = get_program()
    in_maps = make_in_maps(inputs)
    res = run_bass_kernel_spmd(nc, in_maps, core_ids=list(range(NCORES)),
                               trace=trace)
    acc = np.zeros((S, DIM), dtype=np.float32)
    for r in res.results:
        acc += np.asarray(r["out"], dtype=np.float32)
    return acc.reshape(B, S, DIM), res


def kernel(**inputs):
    out, _ = run(inputs, trace=False)
    return out

